# revision 24
# baseline (speedup 1.0000x reference)
"""Causal self-attention (B=1, T=4096, C=1024, H=16) on 8 TRN2 NeuronCores.

Tensor-parallel over heads; core i owns heads (2i, 2i+1) = (A, B).
v2 design (vs the fp32r baseline):
  - bf16 activations/weights end-to-end (PSUM accumulation stays fp32):
    halves DMA traffic and enables DVE 2x/4x modes. y output is bf16;
    the host upcasts and reduces the 8 partial sums in fp64.
  - S^T matmuls for the two heads run as concurrent PE row-tiles:
    head A operands live at partitions 0-63, head B at 64-127 (the natural
    qkv.T layout), so tile_position (0,0)/(64,0) is auto-derived and the two
    K=64 matmuls overlap in the 128x128 array.
  - PV keeps the ones-column trick (stationary [v|1], M=65) so softmax
    denominators fall out of the same accumulation.
  - exp runs mostly on ACT (Exp activation, scale=1/8 fused); a tunable
    fraction of off-diagonal units runs on DVE via a one-instruction
    Schraudolph approximation: bf16bits(exp(x)) ~= int16((A*x + B)/2^16),
    computed by tensor_scalar(mult,add) with int16-converting output.
  - proj contracts both heads in one K=128 matmul (otu is a single
    [128, T] tile, wp a single [128, C] tile).
  - normalization: one K=2 broadcast matmul per q-group builds both heads'
    reciprocal rows into a [128, 512] tile, one tensor_mul scales otu.
  - program order interleaves qkv(tg+1) and proj(g-1) work quanta into
    attention(g)'s unit loop so the in-order PE queue never head-of-line
    stalls behind an ACT-bound S matmul.
  - DMA issue (~1us of sequencer time each) is spread across queues: x
    loads go through the otherwise-idle GPSIMD sequencer, startup weight
    loads fan out over SP+GPSIMD, y stores stay on SP.
"""

import os
import sys

for _p in ("/opt/trn_rl_repo", "/root/.axon_site/_ro/trn_rl_repo"):
    if os.path.isdir(_p) and _p not in sys.path:
        sys.path.insert(0, _p)

import numpy as np
import ml_dtypes

import concourse.bass as bass
import concourse.bacc as bacc
import concourse.mybir as mybir
import concourse.tile as tile
from concourse.bass_utils import run_bass_kernel_spmd

T = 4096
C = 1024
H = 16
HD = 64
NCORES = 8
HPC = H // NCORES  # heads per core = 2
DT = mybir.dt.float32
BF = mybir.dt.bfloat16
I16 = mybir.dt.int16

NG = 8        # q groups of 512
GQ = 512      # q per group
NTC = T // 128  # 32 k-chunks
SKEW = 2      # ST-ahead-of-PV software pipeline depth (in units)

# Schraudolph exp on DVE for od units where (unit_counter % EXP_DVE_MOD)
# == EXP_DVE_PHASE. 0 disables DVE exp entirely.
EXP_DVE_MOD = 2
EXP_DVE_PHASE = 1
ISCALE = 1.0 / 8.0  # 1/sqrt(HD)
_SCH_C = 0.0573060  # zero-mean log-ratio calibration
SCH_A = (1 << 23) / np.log(2.0) * ISCALE / 65536.0
SCH_B = ((127 << 23) - _SCH_C * (1 << 23)) / 65536.0

BF_NP = ml_dtypes.bfloat16


def _build_body(tc, reps=1):
    nc = tc.nc
    xT = nc.dram_tensor("xT", [C, T], BF, kind="ExternalInput").ap()
    wqkvT = nc.dram_tensor("wqkvT", [C, 384], BF, kind="ExternalInput").ap()
    wpT = nc.dram_tensor("wpT", [128, C], BF, kind="ExternalInput").ap()
    maskT = nc.dram_tensor("maskT", [128, 128], BF, kind="ExternalInput").ap()
    ident = nc.dram_tensor("ident", [128, 128], BF, kind="ExternalInput").ap()
    e2 = nc.dram_tensor("e2", [2, 128], BF, kind="ExternalInput").ap()
    y = nc.dram_tensor("y", [T, C], BF, kind="ExternalOutput").ap()

    Exp = mybir.ActivationFunctionType.Exp
    Mult = mybir.AluOpType.mult
    Add = mybir.AluOpType.add

    from contextlib import ExitStack

    with ExitStack() as ctx:
        consts = ctx.enter_context(tc.tile_pool(name="consts", bufs=1))
        qkvsb = ctx.enter_context(tc.tile_pool(name="qkvsb", bufs=1))
        ptpool = ctx.enter_context(tc.tile_pool(name="ptpool", bufs=6))
        psST = ctx.enter_context(tc.tile_pool(name="psST", bufs=2, space="PSUM"))
        psOT = ctx.enter_context(tc.tile_pool(name="psOT", bufs=2, space="PSUM"))
        psQ = ctx.enter_context(tc.tile_pool(name="psQ", bufs=2, space="PSUM"))

        wq_sb = [consts.tile([128, 384], BF, tag=f"wq{cc}", name=f"wq{cc}")
                 for cc in range(8)]
        wp_sb = consts.tile([128, C], BF, tag="wp")
        mask_sb = consts.tile([128, 128], BF, tag="mask")
        id_sb = consts.tile([128, 128], BF, tag="ident")
        e2_sb = consts.tile([2, 128], BF, tag="e2")

        def emit_const_dmas():
            nc.sync.dma_start(wp_sb[:], wpT[:, :])
            nc.sync.dma_start(mask_sb[:], maskT[:, :])
            nc.sync.dma_start(id_sb[:], ident[:, :])
            nc.sync.dma_start(e2_sb[:], e2[:, :])

        # resident activations
        qkvT_sb = qkvsb.tile([128, 2 * T], BF, tag="qkvT")  # q.T | k.T
        vaug_sb = qkvsb.tile([128, NTC * 130], BF, tag="vaug")
        otu_sb = qkvsb.tile([128, T], BF, tag="otu")

        def qap(h, g):
            return qkvT_sb[64 * h:64 * h + 64, g * GQ:(g + 1) * GQ]

        def kap(h, kc):
            lo = T + kc * 128
            return qkvT_sb[64 * h:64 * h + 64, lo:lo + 128]

        for _rep in range(reps):
            repctx = ExitStack()
            xpool = repctx.enter_context(tc.tile_pool(name="xpool", bufs=16))
            vstage = repctx.enter_context(tc.tile_pool(name="vstage", bufs=2))
            stage = repctx.enter_context(tc.tile_pool(name="stage", bufs=2))
            rinpool = repctx.enter_context(tc.tile_pool(name="rin", bufs=8))
            rbpool = repctx.enter_context(tc.tile_pool(name="rbpool", bufs=2))
            ypool = repctx.enter_context(tc.tile_pool(name="ypool", bufs=3))

            if _rep == 0:
                # spread the startup loads across all four DMA-capable
                # sequencers: issue costs ~0.7-1.4us each and would
                # serialize into an ~11us PE-idle ramp on one queue
                dma_engs = [nc.sync, nc.gpsimd]
                for cc in range(8):
                    dma_engs[cc % 2].dma_start(
                        wq_sb[cc][:], wqkvT[cc * 128:(cc + 1) * 128, :])
                # only the ones-columns (64::65) need initializing; a full
                # memset is a 4.4us DVE head-of-line blocker
                nc.vector.memset(vaug_sb[:, 64::65], 1.0)
                emit_const_dmas()

            s2_tiles = [None] * NG  # [2, GQ] sums rows per group
            exp_ctr = [0]

            # ---------------- qkv work-quantum generator for one tg ----
            def qkv_quanta(tg):
                """Yield work quanta (callables already executed) for the
                qkv+v of t-group tg. Each `yield` is one quantum boundary."""
                xts = []
                for cc in range(8):
                    xt = xpool.tile([128, GQ], BF, tag="x")
                    # issue x loads from the otherwise-idle GPSIMD sequencer
                    # (DMA issue costs ~1.4us of sequencer time each); the
                    # first t-group fans out across all queues to shorten
                    # the startup ramp
                    if tg == 0:
                        eng = [nc.gpsimd, nc.sync][cc % 2]
                    else:
                        eng = nc.gpsimd
                    eng.dma_start(
                        xt[:], xT[cc * 128:(cc + 1) * 128,
                                  tg * GQ:(tg + 1) * GQ])
                    xts.append(xt)
                yield
                vst = vstage.tile([128, GQ], BF, tag="vst", name="vst")
                for m in range(3):
                    ps = psQ.tile([128, GQ], DT, tag="psq", name="ps")
                    for cc in range(8):
                        nc.tensor.matmul(
                            ps[:], wq_sb[cc][:, m * 128:(m + 1) * 128],
                            xts[cc][:], start=(cc == 0), stop=(cc == 7))
                    if m == 0:
                        nc.scalar.copy(
                            qkvT_sb[:, tg * GQ:(tg + 1) * GQ], ps[:])
                    elif m == 1:
                        nc.vector.tensor_copy(
                            qkvT_sb[:, T + tg * GQ:T + (tg + 1) * GQ], ps[:])
                    else:
                        nc.vector.tensor_copy(vst[:], ps[:])
                    yield
                for kcl in range(4):
                    kc = tg * 4 + kcl
                    vt = psQ.tile([128, 128], BF, tag="psq", name="vt")
                    nc.tensor.transpose(
                        vt[:], vst[:, kcl * 128:(kcl + 1) * 128], id_sb[:])
                    off = kc * 130
                    nc.vector.tensor_copy(vaug_sb[:, off:off + 64],
                                          vt[:, 0:64])
                    nc.vector.tensor_copy(vaug_sb[:, off + 65:off + 129],
                                          vt[:, 64:128])
                    yield

            # ---------------- proj work-quantum generator for group g --
            def proj_quanta(g):
                rin2 = rinpool.tile([2, GQ], DT, tag="rin", name="rin2")
                scr = rinpool.tile([2, GQ], DT, tag="rin", name="scr")
                nc.vector.reciprocal_approx_accurate(rin2[:], s2_tiles[g][:],
                                                     scr[:])
                rvr2 = rinpool.tile([2, GQ], BF, tag="rvr", name="rvr2")
                nc.vector.tensor_copy(rvr2[:], rin2[:])
                rb = psQ.tile([128, GQ], DT, tag="psq", name="rb")
                nc.tensor.matmul(rb[:], e2_sb[:], rvr2[:],
                                 start=True, stop=True)
                rbs = rbpool.tile([128, GQ], BF, tag="rbs")
                nc.vector.tensor_copy(rbs[:], rb[:])
                yield
                for t2 in range(4):
                    t0 = g * GQ + t2 * 128
                    # normalize per t-slice so the first proj matmul can
                    # start before the whole group is scaled (shorter tail)
                    nc.vector.tensor_mul(otu_sb[:, t0:t0 + 128],
                                         otu_sb[:, t0:t0 + 128],
                                         rbs[:, t2 * 128:(t2 + 1) * 128])
                    ysb = ypool.tile([128, 2 * GQ], BF, tag="ysb")
                    for og in range(2):
                        yp = psQ.tile([128, GQ], DT, tag="psq", name="yp")
                        nc.tensor.matmul(
                            yp[:], otu_sb[:, t0:t0 + 128],
                            wp_sb[:, og * GQ:(og + 1) * GQ],
                            start=True, stop=True)
                        # balance the PSUM-exit copies across DVE and ACT;
                        # in the tail (last group) split 50/50 since DVE is
                        # busy with the normalization chain while ACT idles
                        mod, phase = (2, 1) if g == NG - 1 else (3, 2)
                        dst = ysb[:, og * GQ:(og + 1) * GQ]
                        if (t2 * 2 + og) % mod == phase:
                            nc.scalar.copy(dst, yp[:])
                        else:
                            nc.vector.tensor_copy(dst, yp[:])
                    # one [128, 1024] store per t-slice: halves the y DMA
                    # issue count on the SP sequencer (which also issues the
                    # latency-critical sums DMAs)
                    nc.sync.dma_start(y[t0:t0 + 128, :], ysb[:])
                    yield

            # ---------------- attention for group g (both heads) -------
            def attention(g, bg_quanta):
                """bg_quanta: list of generators; one quantum is pumped
                after each pipeline step so PE always has non-attention
                work queued behind a potentially-stalling S matmul."""
                def pump():
                    while bg_quanta:
                        try:
                            next(bg_quanta[0])
                            return
                        except StopIteration:
                            bg_quanta.pop(0)

                otA = psOT.tile([65, GQ], DT, tag="ot", name="otA")
                otB = psOT.tile([65, GQ], DT, tag="ot", name="otB")
                ots = (otA, otB)

                units = [("od", kc) for kc in range(0, 4 * g, 2)]
                units += [("dg", 4 * g), ("dg", 4 * g + 2)]
                nun = len(units)

                def emit_st(u):
                    kind, kc = u
                    sts, pts = [], []
                    for h in range(HPC):
                        sts.append(psST.tile([128, 2 * GQ], DT, tag="st",
                                             name=f"st{h}"))
                        pts.append(ptpool.tile([128, 2 * GQ], BF, tag="pt",
                                               name=f"pt{h}"))
                    if kind == "od":
                        for i in (0, 1):
                            for h in range(HPC):
                                nc.tensor.matmul(
                                    sts[h][:, i * GQ:(i + 1) * GQ],
                                    kap(h, kc + i), qap(h, g),
                                    start=True, stop=True)
                        use_dve = (
                            EXP_DVE_MOD > 0
                            and exp_ctr[0] % EXP_DVE_MOD == EXP_DVE_PHASE)
                        exp_ctr[0] += 1
                        for h in range(HPC):
                            # on offload units only head B goes to DVE, so
                            # the two heads' exps run on different engines
                            # concurrently instead of serializing on one
                            if use_dve and h == 1:
                                nc.vector.tensor_scalar(
                                    pts[h][:].bitcast(I16), sts[h][:],
                                    float(SCH_A), float(SCH_B), Mult, Add)
                            else:
                                nc.scalar.activation(pts[h][:], sts[h][:],
                                                     Exp, scale=ISCALE)
                    else:
                        j0 = kc - 4 * g
                        nq0 = GQ - 128 * j0
                        nq1 = GQ - 128 * (j0 + 1)
                        for h in range(HPC):
                            nc.tensor.matmul(
                                sts[h][:, 0:nq0], kap(h, kc),
                                qkvT_sb[64 * h:64 * h + 64,
                                        g * GQ + 128 * j0:(g + 1) * GQ],
                                start=True, stop=True)
                            nc.tensor.matmul(
                                sts[h][:, nq0:nq0 + nq1], kap(h, kc + 1),
                                qkvT_sb[64 * h:64 * h + 64,
                                        g * GQ + 128 * (j0 + 1):(g + 1) * GQ],
                                start=True, stop=True)
                        for h in range(HPC):
                            nc.scalar.activation(pts[h][:, 0:nq0 + nq1],
                                                 sts[h][:, 0:nq0 + nq1],
                                                 Exp, scale=ISCALE)
                            nc.vector.tensor_mul(pts[h][:, 0:128],
                                                 pts[h][:, 0:128], mask_sb[:])
                            nc.vector.tensor_mul(pts[h][:, nq0:nq0 + 128],
                                                 pts[h][:, nq0:nq0 + 128],
                                                 mask_sb[:])
                    return pts

                def emit_pv(u, pts, first, last):
                    kind, kc = u
                    if kind == "od":
                        for i in (0, 1):
                            for h in range(HPC):
                                voff = (kc + i) * 130 + h * 65
                                nc.tensor.matmul(
                                    ots[h][0:65, :],
                                    vaug_sb[:, voff:voff + 65],
                                    pts[h][:, i * GQ:(i + 1) * GQ],
                                    start=(first and i == 0),
                                    stop=(last and i == 1))
                    else:
                        j0 = kc - 4 * g
                        nq0 = GQ - 128 * j0
                        nq1 = GQ - 128 * (j0 + 1)
                        for (j, nq, po) in ((j0, nq0, 0), (j0 + 1, nq1, nq0)):
                            for h in range(HPC):
                                voff = (4 * g + j) * 130 + h * 65
                                nc.tensor.matmul(
                                    ots[h][0:65, GQ - nq:GQ],
                                    vaug_sb[:, voff:voff + 65],
                                    pts[h][:, po:po + nq],
                                    start=(first and j == j0),
                                    stop=(last and j == j0 + 1))

                pts_ring = [None] * nun
                for ui, u in enumerate(units):
                    # pump background work BEFORE the (potentially ACT-bound)
                    # S matmuls so the in-order PE queue has work queued
                    # ahead of the stall point
                    pump()
                    pts_ring[ui] = emit_st(u)
                    if ui >= SKEW:
                        k = ui - SKEW
                        emit_pv(units[k], pts_ring[k], k == 0, k == nun - 1)
                        pump()
                for k in range(max(0, nun - SKEW), nun):
                    emit_pv(units[k], pts_ring[k], k == 0, k == nun - 1)
                    pump()

                # sums rows -> s2 tile; otu copies (both heads)
                s2 = rinpool.tile([2, GQ], DT, tag="s2", name="s2")
                for h in range(HPC):
                    sst = stage.tile([65, GQ], DT, tag="sst", name="sst")
                    if h == 0:
                        nc.vector.tensor_copy(sst[64:65, :], ots[h][64:65, :])
                    else:
                        nc.scalar.copy(sst[64:65, :], ots[h][64:65, :])
                    nc.sync.dma_start(s2[h:h + 1, :], sst[64:65, :])
                    nc.vector.tensor_copy(
                        otu_sb[64 * h:64 * h + 64, g * GQ:(g + 1) * GQ],
                        ots[h][0:64, :])
                s2_tiles[g] = s2
                while bg_quanta:
                    pump()

            # ---------------- main schedule ----------------------------
            pending = [qkv_quanta(0)]
            while pending:
                try:
                    next(pending[0])
                except StopIteration:
                    pending.pop(0)
            for g in range(NG):
                bg = []
                if g + 1 < NG:
                    bg.append(qkv_quanta(g + 1))
                if g >= 1:
                    bg.append(proj_quanta(g - 1))
                attention(g, bg)
            for _ in proj_quanta(NG - 1):
                pass
            repctx.close()


_CACHE = {}


def build_module(reps=1):
    key = ("nc", reps)
    if key not in _CACHE:
        nc = bacc.Bacc("TRN2", target_bir_lowering=False, debug=False,
                       num_swdge_queues=4)
        with tile.TileContext(nc) as tc:
            _build_body(tc, reps=reps)
        nc.compile()
        _CACHE[key] = nc
    return _CACHE[key]


def _host_prep(x, w_attn, w_proj):
    x = np.asarray(x, dtype=np.float32)
    w_attn = np.asarray(w_attn, dtype=np.float32)
    w_proj = np.asarray(w_proj, dtype=np.float32)
    X = x.reshape(T, C)
    xTh = np.ascontiguousarray(X.T).astype(BF_NP)
    mask = np.triu(np.ones((128, 128), dtype=np.float32)).astype(BF_NP)
    eye = np.eye(128, dtype=np.float32).astype(BF_NP)
    e2h = np.zeros((2, 128), dtype=np.float32)
    e2h[0, 0:64] = 1.0
    e2h[1, 64:128] = 1.0
    e2h = e2h.astype(BF_NP)
    Wq, Wk, Wv = w_attn[0:C], w_attn[C:2 * C], w_attn[2 * C:3 * C]
    in_maps = []
    for i in range(NCORES):
        hA, hB = 2 * i, 2 * i + 1
        Wc = np.concatenate([
            Wq[64 * hA:64 * hA + 64], Wq[64 * hB:64 * hB + 64],
            Wk[64 * hA:64 * hA + 64], Wk[64 * hB:64 * hB + 64],
            Wv[64 * hA:64 * hA + 64], Wv[64 * hB:64 * hB + 64],
        ], axis=0)  # [384, C]
        in_maps.append({
            "xT": xTh,
            "wqkvT": np.ascontiguousarray(Wc.T).astype(BF_NP),
            "wpT": np.ascontiguousarray(
                w_proj[:, 128 * i:128 * (i + 1)].T).astype(BF_NP),
            "maskT": mask,
            "ident": eye,
            "e2": e2h,
        })
    return in_maps


def run(x, w_attn, w_proj, trace=False):
    nc = build_module()
    in_maps = _host_prep(x, w_attn, w_proj)
    res = run_bass_kernel_spmd(nc, in_maps, core_ids=list(range(NCORES)),
                               trace=trace)
    parts = np.stack([r["y"].astype(np.float64) for r in res.results], axis=0)
    yfull = parts.sum(axis=0).astype(np.float32)
    return yfull.reshape(1, T, C), res


def kernel(x, w_attn, w_proj):
    yfull, _ = run(x, w_attn, w_proj, trace=False)
    return yfull


# revision 28
# speedup vs baseline: 1.0722x; 1.0722x over previous
"""Causal self-attention (B=1, T=4096, C=1024, H=16) on 8 TRN2 NeuronCores.

Tensor-parallel over heads; core i owns heads (2i, 2i+1) = (A, B).
v2 design (vs the fp32r baseline):
  - bf16 activations/weights end-to-end (PSUM accumulation stays fp32):
    halves DMA traffic and enables DVE 2x/4x modes. y output is bf16;
    the host upcasts and reduces the 8 partial sums in fp64.
  - S^T matmuls for the two heads run as concurrent PE row-tiles:
    head A operands live at partitions 0-63, head B at 64-127 (the natural
    qkv.T layout), so tile_position (0,0)/(64,0) is auto-derived and the two
    K=64 matmuls overlap in the 128x128 array.
  - PV keeps the ones-column trick (stationary [v|1], M=65) so softmax
    denominators fall out of the same accumulation.
  - exp runs mostly on ACT (Exp activation, scale=1/8 fused); a tunable
    fraction of off-diagonal units runs on DVE via a one-instruction
    Schraudolph approximation: bf16bits(exp(x)) ~= int16((A*x + B)/2^16),
    computed by tensor_scalar(mult,add) with int16-converting output.
  - proj contracts both heads in one K=128 matmul (otu is a single
    [128, T] tile, wp a single [128, C] tile).
  - normalization: one K=2 broadcast matmul per q-group builds both heads'
    reciprocal rows into a [128, 512] tile, one tensor_mul scales otu.
  - program order interleaves qkv(tg+1) and proj(g-1) work quanta into
    attention(g)'s unit loop so the in-order PE queue never head-of-line
    stalls behind an ACT-bound S matmul.
  - DMA issue (~1us of sequencer time each) is spread across queues: x
    loads go through the otherwise-idle GPSIMD sequencer, startup weight
    loads fan out over SP+GPSIMD, y stores stay on SP.
"""

import os
import sys

for _p in ("/opt/trn_rl_repo", "/root/.axon_site/_ro/trn_rl_repo"):
    if os.path.isdir(_p) and _p not in sys.path:
        sys.path.insert(0, _p)

import numpy as np
import ml_dtypes

import concourse.bass as bass
import concourse.bacc as bacc
import concourse.mybir as mybir
import concourse.tile as tile
from concourse.bass_utils import run_bass_kernel_spmd

T = 4096
C = 1024
H = 16
HD = 64
NCORES = 8
HPC = H // NCORES  # heads per core = 2
DT = mybir.dt.float32
BF = mybir.dt.bfloat16
I16 = mybir.dt.int16

NG = 8        # q groups of 512
GQ = 512      # q per group
NTC = T // 128  # 32 k-chunks
SKEW = 2      # ST-ahead-of-PV software pipeline depth (in units)

# Schraudolph exp on DVE for od units where (unit_counter % EXP_DVE_MOD)
# == EXP_DVE_PHASE. 0 disables DVE exp entirely.
EXP_DVE_MOD = 2
EXP_DVE_PHASE = 1
ISCALE = 1.0 / 8.0  # 1/sqrt(HD)
_SCH_C = 0.0573060  # zero-mean log-ratio calibration
SCH_A = (1 << 23) / np.log(2.0) * ISCALE / 65536.0
SCH_B = ((127 << 23) - _SCH_C * (1 << 23)) / 65536.0

BF_NP = ml_dtypes.bfloat16


def _build_body(tc, reps=1):
    nc = tc.nc
    xT = nc.dram_tensor("xT", [C, T], BF, kind="ExternalInput").ap()
    wqkvT = nc.dram_tensor("wqkvT", [C, 384], BF, kind="ExternalInput").ap()
    wpT = nc.dram_tensor("wpT", [128, C], BF, kind="ExternalInput").ap()
    maskT = nc.dram_tensor("maskT", [128, 128], BF, kind="ExternalInput").ap()
    ident = nc.dram_tensor("ident", [128, 128], BF, kind="ExternalInput").ap()
    e2 = nc.dram_tensor("e2", [2, 128], BF, kind="ExternalInput").ap()
    y = nc.dram_tensor("y", [T, C], BF, kind="ExternalOutput").ap()

    Exp = mybir.ActivationFunctionType.Exp
    Mult = mybir.AluOpType.mult
    Add = mybir.AluOpType.add

    from contextlib import ExitStack

    with ExitStack() as ctx:
        consts = ctx.enter_context(tc.tile_pool(name="consts", bufs=1))
        qkvsb = ctx.enter_context(tc.tile_pool(name="qkvsb", bufs=1))
        ptpool = ctx.enter_context(tc.tile_pool(name="ptpool", bufs=6))
        psST = ctx.enter_context(tc.tile_pool(name="psST", bufs=2, space="PSUM"))
        psOT = ctx.enter_context(tc.tile_pool(name="psOT", bufs=2, space="PSUM"))
        psQ = ctx.enter_context(tc.tile_pool(name="psQ", bufs=2, space="PSUM"))

        wq_sb = [consts.tile([128, 384], BF, tag=f"wq{cc}", name=f"wq{cc}")
                 for cc in range(8)]
        wp_sb = consts.tile([128, C], BF, tag="wp")
        mask_sb = consts.tile([128, 128], BF, tag="mask")
        id_sb = consts.tile([128, 128], BF, tag="ident")
        e2_sb = consts.tile([2, 128], BF, tag="e2")

        def emit_const_dmas():
            nc.sync.dma_start(wp_sb[:], wpT[:, :])
            nc.sync.dma_start(mask_sb[:], maskT[:, :])
            nc.sync.dma_start(id_sb[:], ident[:, :])
            nc.sync.dma_start(e2_sb[:], e2[:, :])

        # resident activations
        qkvT_sb = qkvsb.tile([128, 2 * T], BF, tag="qkvT")  # q.T | k.T
        vaug_sb = qkvsb.tile([128, NTC * 130], BF, tag="vaug")
        otu_sb = qkvsb.tile([128, T], BF, tag="otu")

        def qap(h, g):
            return qkvT_sb[64 * h:64 * h + 64, g * GQ:(g + 1) * GQ]

        def kap(h, kc):
            lo = T + kc * 128
            return qkvT_sb[64 * h:64 * h + 64, lo:lo + 128]

        for _rep in range(reps):
            repctx = ExitStack()
            xpool = repctx.enter_context(tc.tile_pool(name="xpool", bufs=16))
            vstage = repctx.enter_context(tc.tile_pool(name="vstage", bufs=2))
            stage = repctx.enter_context(tc.tile_pool(name="stage", bufs=2))
            rinpool = repctx.enter_context(tc.tile_pool(name="rin", bufs=8))
            rbpool = repctx.enter_context(tc.tile_pool(name="rbpool", bufs=2))
            ypool = repctx.enter_context(tc.tile_pool(name="ypool", bufs=3))

            if _rep == 0:
                # spread the startup loads across all four DMA-capable
                # sequencers: issue costs ~0.7-1.4us each and would
                # serialize into an ~11us PE-idle ramp on one queue
                dma_engs = [nc.sync, nc.gpsimd]
                for cc in range(8):
                    dma_engs[cc % 2].dma_start(
                        wq_sb[cc][:], wqkvT[cc * 128:(cc + 1) * 128, :])
                # only the ones-columns (64::65) need initializing; a full
                # memset is a 4.4us DVE head-of-line blocker
                nc.vector.memset(vaug_sb[:, 64::65], 1.0)
                emit_const_dmas()

            s2_tiles = [None] * NG  # [2, GQ] sums rows per group
            exp_ctr = [0]

            # ---------------- qkv work-quantum generator for one tg ----
            def qkv_quanta(tg):
                """Yield work quanta (callables already executed) for the
                qkv+v of t-group tg. Each `yield` is one quantum boundary."""
                xts = []
                for cc in range(8):
                    xt = xpool.tile([128, GQ], BF, tag="x")
                    # issue x loads from the otherwise-idle GPSIMD sequencer
                    # (DMA issue costs ~1.4us of sequencer time each); the
                    # first t-group fans out across all queues to shorten
                    # the startup ramp
                    if tg == 0:
                        eng = [nc.gpsimd, nc.sync][cc % 2]
                    else:
                        eng = nc.gpsimd
                    eng.dma_start(
                        xt[:], xT[cc * 128:(cc + 1) * 128,
                                  tg * GQ:(tg + 1) * GQ])
                    xts.append(xt)
                yield
                vst = vstage.tile([128, GQ], BF, tag="vst", name="vst")
                for m in range(3):
                    ps = psQ.tile([128, GQ], DT, tag="psq", name="ps")
                    for cc in range(8):
                        nc.tensor.matmul(
                            ps[:], wq_sb[cc][:, m * 128:(m + 1) * 128],
                            xts[cc][:], start=(cc == 0), stop=(cc == 7))
                    if m == 0:
                        nc.scalar.copy(
                            qkvT_sb[:, tg * GQ:(tg + 1) * GQ], ps[:])
                    elif m == 1:
                        nc.vector.tensor_copy(
                            qkvT_sb[:, T + tg * GQ:T + (tg + 1) * GQ], ps[:])
                    else:
                        nc.vector.tensor_copy(vst[:], ps[:])
                    yield
                for kcl in range(4):
                    kc = tg * 4 + kcl
                    vt = psQ.tile([128, 128], BF, tag="psq", name="vt")
                    nc.tensor.transpose(
                        vt[:], vst[:, kcl * 128:(kcl + 1) * 128], id_sb[:])
                    off = kc * 130
                    nc.vector.tensor_copy(vaug_sb[:, off:off + 64],
                                          vt[:, 0:64])
                    nc.vector.tensor_copy(vaug_sb[:, off + 65:off + 129],
                                          vt[:, 64:128])
                    yield

            # ---------------- proj work-quantum generator for group g --
            def proj_quanta(g):
                rin2 = rinpool.tile([2, GQ], DT, tag="rin", name="rin2")
                scr = rinpool.tile([2, GQ], DT, tag="rin", name="scr")
                nc.vector.reciprocal_approx_accurate(rin2[:], s2_tiles[g][:],
                                                     scr[:])
                rvr2 = rinpool.tile([2, GQ], BF, tag="rvr", name="rvr2")
                nc.vector.tensor_copy(rvr2[:], rin2[:])
                rb = psQ.tile([128, GQ], DT, tag="psq", name="rb")
                nc.tensor.matmul(rb[:], e2_sb[:], rvr2[:],
                                 start=True, stop=True)
                rbs = rbpool.tile([128, GQ], BF, tag="rbs")
                nc.vector.tensor_copy(rbs[:], rb[:])
                yield
                for t2 in range(4):
                    t0 = g * GQ + t2 * 128
                    # normalize per t-slice so the first proj matmul can
                    # start before the whole group is scaled (shorter tail)
                    nc.vector.tensor_mul(otu_sb[:, t0:t0 + 128],
                                         otu_sb[:, t0:t0 + 128],
                                         rbs[:, t2 * 128:(t2 + 1) * 128])
                    ysb = ypool.tile([128, 2 * GQ], BF, tag="ysb")
                    for og in range(2):
                        yp = psQ.tile([128, GQ], DT, tag="psq", name="yp")
                        nc.tensor.matmul(
                            yp[:], otu_sb[:, t0:t0 + 128],
                            wp_sb[:, og * GQ:(og + 1) * GQ],
                            start=True, stop=True)
                        # balance the PSUM-exit copies across DVE and ACT;
                        # in the tail (last group) split 50/50 since DVE is
                        # busy with the normalization chain while ACT idles
                        mod, phase = (2, 1) if g == NG - 1 else (3, 2)
                        dst = ysb[:, og * GQ:(og + 1) * GQ]
                        if (t2 * 2 + og) % mod == phase:
                            nc.scalar.copy(dst, yp[:])
                        else:
                            nc.vector.tensor_copy(dst, yp[:])
                    # one [128, 1024] store per t-slice: halves the y DMA
                    # issue count on the SP sequencer (which also issues the
                    # latency-critical sums DMAs)
                    nc.sync.dma_start(y[t0:t0 + 128, :], ysb[:])
                    yield

            # ---------------- attention for group g (both heads) -------
            def attention(g, bg_quanta):
                """bg_quanta: list of generators; one quantum is pumped
                after each pipeline step so PE always has non-attention
                work queued behind a potentially-stalling S matmul."""
                def pump():
                    while bg_quanta:
                        try:
                            next(bg_quanta[0])
                            return
                        except StopIteration:
                            bg_quanta.pop(0)

                otA = psOT.tile([65, GQ], DT, tag="ot", name="otA")
                otB = psOT.tile([65, GQ], DT, tag="ot", name="otB")
                ots = (otA, otB)

                units = [("od", kc) for kc in range(0, 4 * g, 2)]
                units += [("dg", 4 * g), ("dg", 4 * g + 2)]
                nun = len(units)

                def emit_st(u):
                    kind, kc = u
                    sts, pts = [], []
                    for h in range(HPC):
                        sts.append(psST.tile([128, 2 * GQ], DT, tag="st",
                                             name=f"st{h}"))
                        pts.append(ptpool.tile([128, 2 * GQ], BF, tag="pt",
                                               name=f"pt{h}"))
                    if kind == "od":
                        for i in (0, 1):
                            for h in range(HPC):
                                nc.tensor.matmul(
                                    sts[h][:, i * GQ:(i + 1) * GQ],
                                    kap(h, kc + i), qap(h, g),
                                    start=True, stop=True)
                        use_dve = (
                            EXP_DVE_MOD > 0
                            and exp_ctr[0] % EXP_DVE_MOD == EXP_DVE_PHASE)
                        exp_ctr[0] += 1
                        for h in range(HPC):
                            # on offload units only head B goes to DVE, so
                            # the two heads' exps run on different engines
                            # concurrently instead of serializing on one
                            if use_dve and h == 1:
                                nc.vector.tensor_scalar(
                                    pts[h][:].bitcast(I16), sts[h][:],
                                    float(SCH_A), float(SCH_B), Mult, Add)
                            else:
                                nc.scalar.activation(pts[h][:], sts[h][:],
                                                     Exp, scale=ISCALE)
                    else:
                        j0 = kc - 4 * g
                        nq0 = GQ - 128 * j0
                        nq1 = GQ - 128 * (j0 + 1)
                        for h in range(HPC):
                            nc.tensor.matmul(
                                sts[h][:, 0:nq0], kap(h, kc),
                                qkvT_sb[64 * h:64 * h + 64,
                                        g * GQ + 128 * j0:(g + 1) * GQ],
                                start=True, stop=True)
                            nc.tensor.matmul(
                                sts[h][:, nq0:nq0 + nq1], kap(h, kc + 1),
                                qkvT_sb[64 * h:64 * h + 64,
                                        g * GQ + 128 * (j0 + 1):(g + 1) * GQ],
                                start=True, stop=True)
                        for h in range(HPC):
                            nc.scalar.activation(pts[h][:, 0:nq0 + nq1],
                                                 sts[h][:, 0:nq0 + nq1],
                                                 Exp, scale=ISCALE)
                            nc.vector.tensor_mul(pts[h][:, 0:128],
                                                 pts[h][:, 0:128], mask_sb[:])
                            nc.vector.tensor_mul(pts[h][:, nq0:nq0 + 128],
                                                 pts[h][:, nq0:nq0 + 128],
                                                 mask_sb[:])
                    return pts

                def emit_pv(u, pts, first, last):
                    kind, kc = u
                    if kind == "od":
                        for i in (0, 1):
                            for h in range(HPC):
                                voff = (kc + i) * 130 + h * 65
                                nc.tensor.matmul(
                                    ots[h][0:65, :],
                                    vaug_sb[:, voff:voff + 65],
                                    pts[h][:, i * GQ:(i + 1) * GQ],
                                    start=(first and i == 0),
                                    stop=(last and i == 1))
                    else:
                        j0 = kc - 4 * g
                        nq0 = GQ - 128 * j0
                        nq1 = GQ - 128 * (j0 + 1)
                        for (j, nq, po) in ((j0, nq0, 0), (j0 + 1, nq1, nq0)):
                            for h in range(HPC):
                                voff = (4 * g + j) * 130 + h * 65
                                nc.tensor.matmul(
                                    ots[h][0:65, GQ - nq:GQ],
                                    vaug_sb[:, voff:voff + 65],
                                    pts[h][:, po:po + nq],
                                    start=(first and j == j0),
                                    stop=(last and j == j0 + 1))

                pts_ring = [None] * nun
                for ui, u in enumerate(units):
                    # pump background work BEFORE the (potentially ACT-bound)
                    # S matmuls so the in-order PE queue has work queued
                    # ahead of the stall point
                    pump()
                    pts_ring[ui] = emit_st(u)
                    if ui >= SKEW:
                        k = ui - SKEW
                        emit_pv(units[k], pts_ring[k], k == 0, k == nun - 1)
                        pump()
                for k in range(max(0, nun - SKEW), nun):
                    emit_pv(units[k], pts_ring[k], k == 0, k == nun - 1)
                    pump()

                # sums rows -> s2 tile; otu copies (both heads)
                s2 = rinpool.tile([2, GQ], DT, tag="s2", name="s2")
                for h in range(HPC):
                    sst = stage.tile([65, GQ], DT, tag="sst", name="sst")
                    if h == 0:
                        nc.vector.tensor_copy(sst[64:65, :], ots[h][64:65, :])
                    else:
                        nc.scalar.copy(sst[64:65, :], ots[h][64:65, :])
                    nc.sync.dma_start(s2[h:h + 1, :], sst[64:65, :])
                    nc.vector.tensor_copy(
                        otu_sb[64 * h:64 * h + 64, g * GQ:(g + 1) * GQ],
                        ots[h][0:64, :])
                s2_tiles[g] = s2
                while bg_quanta:
                    pump()

            # ---------------- main schedule ----------------------------
            pending = [qkv_quanta(0)]
            while pending:
                try:
                    next(pending[0])
                except StopIteration:
                    pending.pop(0)
            for g in range(NG):
                bg = []
                if g + 1 < NG:
                    bg.append(qkv_quanta(g + 1))
                if g >= 1:
                    bg.append(proj_quanta(g - 1))
                attention(g, bg)
            for _ in proj_quanta(NG - 1):
                pass
            repctx.close()


_CACHE = {}


def build_module(reps=1):
    key = ("nc", reps)
    if key not in _CACHE:
        nc = bacc.Bacc("TRN2", target_bir_lowering=False, debug=False,
                       num_swdge_queues=4)
        with tile.TileContext(nc) as tc:
            _build_body(tc, reps=reps)
        nc.compile()
        _CACHE[key] = nc
    return _CACHE[key]


def _host_prep(x, w_attn, w_proj):
    x = np.asarray(x, dtype=np.float32)
    w_attn = np.asarray(w_attn, dtype=np.float32)
    w_proj = np.asarray(w_proj, dtype=np.float32)
    X = x.reshape(T, C)
    xTh = np.ascontiguousarray(X.T).astype(BF_NP)
    mask = np.triu(np.ones((128, 128), dtype=np.float32)).astype(BF_NP)
    eye = np.eye(128, dtype=np.float32).astype(BF_NP)
    e2h = np.zeros((2, 128), dtype=np.float32)
    e2h[0, 0:64] = 1.0
    e2h[1, 64:128] = 1.0
    e2h = e2h.astype(BF_NP)
    Wq, Wk, Wv = w_attn[0:C], w_attn[C:2 * C], w_attn[2 * C:3 * C]
    in_maps = []
    for i in range(NCORES):
        hA, hB = 2 * i, 2 * i + 1
        Wc = np.concatenate([
            Wq[64 * hA:64 * hA + 64], Wq[64 * hB:64 * hB + 64],
            Wk[64 * hA:64 * hA + 64], Wk[64 * hB:64 * hB + 64],
            Wv[64 * hA:64 * hA + 64], Wv[64 * hB:64 * hB + 64],
        ], axis=0)  # [384, C]
        in_maps.append({
            "xT": xTh,
            "wqkvT": np.ascontiguousarray(Wc.T).astype(BF_NP),
            "wpT": np.ascontiguousarray(
                w_proj[:, 128 * i:128 * (i + 1)].T).astype(BF_NP),
            "maskT": mask,
            "ident": eye,
            "e2": e2h,
        })
    return in_maps


def run(x, w_attn, w_proj, trace=False):
    nc = build_module()
    in_maps = _host_prep(x, w_attn, w_proj)
    res = run_bass_kernel_spmd(nc, in_maps, core_ids=list(range(NCORES)),
                               trace=trace)
    parts = np.stack([r["y"].astype(np.float64) for r in res.results], axis=0)
    yfull = parts.sum(axis=0).astype(np.float32)
    return yfull.reshape(1, T, C), res


def kernel(x, w_attn, w_proj):
    yfull, _ = run(x, w_attn, w_proj, trace=False)
    return yfull
